# revision 36
# baseline (speedup 1.0000x reference)
"""Multi-head attention (B=4, S=2048, D=1024, H=16, causal) on 8 trn2 NeuronCores.

Sharding: tensor-parallel over heads. Core c owns heads {2c, 2c+1} = model dims
[c*128, (c+1)*128).

Per-core pipeline (all matmul inputs bf16, fp32 PSUM accumulation):
  A) Q/K/V projections in transposed layout  X_c [128 dims, rows]
     (lhsT = W.T chunk stationary, rhs = host-pretransposed input stream).
     f-outer loop order: each weight chunk loaded once per 2 psum groups.
  B) V transposed back to row-major via XBAR DMA transpose into
     v_aug [128 k-rows, kblk, 192]: cols 0:128 = [h0 dims | h1 dims],
     cols 128:192 = ones. PV lhsT uses a strided AP selecting
     [h_dims 64 | ones 64] so PSUM rows carry context + the softmax
     denominator replicated 64x.
  C) Attention per (batch, 512-q window, 128-k block), causal blocks only:
     scoresT [k,q] via 2-head row-packed matmuls (column-trimmed on the
     diagonal), exp (scale=1/8 folded in, no max subtraction - scores are
     O(1) by construction), triangular mask on diagonal blocks, PV
     accumulation per head.  Softmax division via reciprocal_approx_fast
     (DVE custom op) + tensor_mul -> ctxT [dims, q] bf16.
  D) Output projection partials -> outT [1024 o, q] bf16 per window; host
     sums the 8 cores' partials in fp32, transposes, adds bo.
"""

import os
import sys
from contextlib import ExitStack

sys.path.insert(0, "/opt/trn_rl_repo")

import numpy as np
import ml_dtypes

import concourse.bass as bass
import concourse.bacc as bacc
import concourse.mybir as mybir
import concourse.tile as tile
from concourse.bass_utils import run_bass_kernel_spmd

BF = mybir.dt.bfloat16
F32 = mybir.dt.float32
bf16 = ml_dtypes.bfloat16

B, S, D = 4, 2048, 1024
R = B * S  # 8192
NCORES = 8
QW = 512  # q-window
NKB = S // 128  # 16 k-blocks per batch

_CACHE: dict = {}


def _build_program() -> bass.Bass:
    nc = bacc.Bacc(None, num_devices=NCORES)
    xqT = nc.dram_tensor("xqT", [D, R], BF, kind="ExternalInput")
    xkT = nc.dram_tensor("xkT", [D, R], BF, kind="ExternalInput")
    xvT = nc.dram_tensor("xvT", [D, R], BF, kind="ExternalInput")
    # host pre-rearranged to [128, 8, 128] so the load is one contiguous DMA
    wq = nc.dram_tensor("wq", [128, 8, 128], BF, kind="ExternalInput")
    wk = nc.dram_tensor("wk", [128, 8, 128], BF, kind="ExternalInput")
    wv = nc.dram_tensor("wv", [128, 8, 128], BF, kind="ExternalInput")
    wo = nc.dram_tensor("wo", [128, D], BF, kind="ExternalInput")
    bq = nc.dram_tensor("bq", [128, 1], F32, kind="ExternalInput")
    bk = nc.dram_tensor("bk", [128, 1], F32, kind="ExternalInput")
    bv = nc.dram_tensor("bv", [128, 1], F32, kind="ExternalInput")
    tri = nc.dram_tensor("tri", [128, 2, 128], BF, kind="ExternalInput")
    ident = nc.dram_tensor("ident", [128, 128], BF, kind="ExternalInput")
    out_ext = nc.dram_tensor("out", [D, R], BF, kind="ExternalOutput")

    EXP = mybir.ActivationFunctionType.Exp

    with ExitStack() as ctx:
        tc = ctx.enter_context(tile.TileContext(nc))
        singles = ctx.enter_context(tc.tile_pool(name="singles", bufs=1))
        stage = ctx.enter_context(tc.tile_pool(name="stage", bufs=36))
        qkv = ctx.enter_context(tc.tile_pool(name="qkv", bufs=4))
        vst = ctx.enter_context(tc.tile_pool(name="vst", bufs=3))
        exps = ctx.enter_context(tc.tile_pool(name="exps", bufs=8))
        divp = ctx.enter_context(tc.tile_pool(name="divp", bufs=4))
        outp = ctx.enter_context(tc.tile_pool(name="outp", bufs=6))
        ps_proj = ctx.enter_context(tc.tile_pool(name="ps_proj", bufs=2, space="PSUM"))
        ps_sc = ctx.enter_context(tc.tile_pool(name="ps_sc", bufs=2, space="PSUM"))
        ps_pv = ctx.enter_context(tc.tile_pool(name="ps_pv", bufs=2, space="PSUM"))

        # resident constants
        wq_sb = singles.tile([128, 8, 128], BF, name="wq_sb")
        wk_sb = singles.tile([128, 8, 128], BF, name="wk_sb")
        wv_sb = singles.tile([128, 8, 128], BF, name="wv_sb")
        nc.sync.dma_start(wq_sb[:], wq[:, :, :])
        nc.sync.dma_start(wk_sb[:], wk[:, :, :])
        nc.sync.dma_start(wv_sb[:], wv[:, :, :])
        wo_sb = singles.tile([128, D], BF, name="wo_sb")
        nc.sync.dma_start(wo_sb[:], wo[:, :])
        bq_sb = singles.tile([128, 1], F32, name="bq_sb")
        bk_sb = singles.tile([128, 1], F32, name="bk_sb")
        bv_sb = singles.tile([128, 1], F32, name="bv_sb")
        nc.sync.dma_start(bq_sb[:], bq[:, :])
        nc.sync.dma_start(bk_sb[:], bk[:, :])
        nc.sync.dma_start(bv_sb[:], bv[:, :])
        tri_sb = singles.tile([128, 2, 128], BF, name="tri_sb")
        nc.sync.dma_start(tri_sb[:], tri[:, :, :])
        id_sb = singles.tile([128, 128], BF, name="id_sb")
        nc.sync.dma_start(id_sb[:], ident[:, :])

        warm_sb = singles.tile([128, 512], BF, name="warm_sb")
        nc.vector.memset(warm_sb[:], 0.0)
        warm_ps = ps_proj.tile([128, 512], F32, tag="proj", name="warm_ps")
        for wi in range(10):
            nc.tensor.matmul(
                warm_ps[:],
                warm_sb[:, 0:128],
                warm_sb[:],
                start=(wi == 0),
                stop=(wi == 9),
            )

        tiles = {}

        def alloc_batch(b):
            q_sb = qkv.tile([128, S], BF, tag="q_sb", name=f"q_sb{b}")
            k_sb = qkv.tile([128, S], BF, tag="k_sb", name=f"k_sb{b}")
            # [h0 dims (0:64) | ones (64:128) | h1 dims (128:192)]
            v_aug = qkv.tile([128, NKB, 192], BF, tag="v_aug", name=f"v_aug{b}")
            nc.vector.memset(v_aug[:, :, 64:128], 1.0)
            tiles[b] = (q_sb, k_sb, v_aug)

        def emit_projection_dmas(b, which, nchunks=1):
            # which: 0=q, 1=k, 2=v.  Issues the staged input DMAs.  nchunks>1
            # splits each tile's load so the first matmuls can start before
            # the whole tile has landed (single transfers run on one DMA
            # engine at ~20GB/s, so a full 256KB tile takes ~13us).
            if b not in tiles:
                alloc_batch(b)
            xT = (xqT, xkT, xvT)[which]
            st = {}
            cw = (S // 2) // nchunks
            for half in range(2):
                for f in range(8):
                    s_t = stage.tile([128, S // 2], BF, tag="stage")
                    base = b * S + half * (S // 2)
                    for c in range(nchunks):
                        eng = nc.sync if (f + c) % 2 == 0 else nc.gpsimd
                        eng.dma_start(
                            s_t[:, c * cw : (c + 1) * cw],
                            xT[
                                f * 128 : (f + 1) * 128,
                                base + c * cw : base + (c + 1) * cw,
                            ],
                        )
                    st[(f, half)] = s_t
            return st

        def emit_projection_mms(b, which, st):
            # Emits the 2x2 psum groups + evac, consuming staged tiles.
            q_sb, k_sb, v_aug = tiles[b]
            w_sb, b_sb = (
                (wq_sb, bq_sb),
                (wk_sb, bk_sb),
                (wv_sb, bv_sb),
            )[which]

            def evac(t, ps):
                if which == 0:
                    nc.vector.tensor_scalar_add(
                        q_sb[:, t * 512 : (t + 1) * 512], ps[:], bq_sb[:]
                    )
                elif which == 1:
                    nc.vector.tensor_scalar_add(
                        k_sb[:, t * 512 : (t + 1) * 512], ps[:], bk_sb[:]
                    )
                else:
                    v_st = vst.tile([128, 512], BF, tag="v_st")
                    nc.vector.tensor_scalar_add(v_st[:], ps[:], bv_sb[:])
                    for s4 in range(4):
                        pst = ps_proj.tile(
                            [128, 128], BF, tag="proj", name=f"pst{b}_{t}_{s4}"
                        )
                        nc.tensor.transpose(
                            pst[:], v_st[:, s4 * 128 : (s4 + 1) * 128], id_sb[:]
                        )
                        tt = t * 4 + s4
                        nc.vector.tensor_copy(v_aug[:, tt, 0:64], pst[:, 0:64])
                        nc.vector.tensor_copy(v_aug[:, tt, 128:192], pst[:, 64:128])

            for half in range(2):
                ps0 = ps_proj.tile(
                    [128, 512], F32, tag="proj", name=f"ps{b}_{half}a_{which}"
                )
                ps1 = ps_proj.tile(
                    [128, 512], F32, tag="proj", name=f"ps{b}_{half}b_{which}"
                )
                for f in range(8):
                    nc.tensor.matmul(
                        ps0[:],
                        w_sb[:, f, :],
                        st[(f, half)][:, 0:512],
                        start=(f == 0),
                        stop=(f == 7),
                    )
                    nc.tensor.matmul(
                        ps1[:],
                        w_sb[:, f, :],
                        st[(f, half)][:, 512:1024],
                        start=(f == 0),
                        stop=(f == 7),
                    )
                evac(half * 2, ps0)
                evac(half * 2 + 1, ps1)

        pending_outproj = []  # (ctx_t, win) whose out-projection is deferred

        def emit_attention_qb(b, qb, prev_outproj=()):
            # Software-pipelined: PV for block k is emitted after scores for
            # block k+2, so the PE never waits on the ACT exp of block k.
            # The previous window's out-projection is woven in after the
            # first two score blocks, filling the divide-latency bubble.
            q_sb, k_sb, v_aug = tiles[b]
            nk = 4 * qb + 4  # causal: k-blocks 0 .. 4qb+3
            pv0 = ps_pv.tile([128, 512], F32, tag="pv", name=f"pv0_{b}_{qb}")
            pv1 = ps_pv.tile([128, 512], F32, tag="pv", name=f"pv1_{b}_{qb}")
            ets = {}

            def emit_scores(kblk):
                r = kblk - 4 * qb
                q_lo = max(0, r * 128)
                sc = ps_sc.tile([128, 2, 512], F32, tag="sc")
                for h in range(2):
                    nc.tensor.matmul(
                        sc[:, h, q_lo:512],
                        k_sb[h * 64 : (h + 1) * 64, kblk * 128 : (kblk + 1) * 128],
                        q_sb[h * 64 : (h + 1) * 64, qb * 512 + q_lo : (qb + 1) * 512],
                        start=True,
                        stop=True,
                        tile_position=(h * 64, 0),
                    )
                et = exps.tile([128, 2, 512], BF, tag="et")
                nc.scalar.activation(
                    et[:, :, q_lo:512], sc[:, :, q_lo:512], EXP, scale=0.125
                )
                if r >= 0:
                    nc.vector.tensor_mul(
                        et[:, :, q_lo : q_lo + 128],
                        et[:, :, q_lo : q_lo + 128],
                        tri_sb[:],
                    )
                ets[kblk] = (et, q_lo)

            def emit_pv(kblk):
                et, q_lo = ets.pop(kblk)
                for h, pv in ((0, pv0), (1, pv1)):
                    nc.tensor.matmul(
                        pv[:, q_lo:512],
                        v_aug[:, kblk, h * 64 : h * 64 + 128],
                        et[:, h, q_lo:512],
                        start=(kblk == 0),
                        stop=(kblk == nk - 1),
                    )

            for kblk in range(nk):
                emit_scores(kblk)
                if kblk >= 2:
                    emit_pv(kblk - 2)
            emit_pv(nk - 2)
            emit_pv(nk - 1)

            # normalize: pv0 rows[0:64]=ctx~ h0, rows[64:128]=l h0 (replicated)
            #            pv1 rows[0:64]=l h1,   rows[64:128]=ctx~ h1
            # reciprocal_approx_fast only works on full-128-partition SBUF
            # tiles, so gather [l0 | l1] into lt first.
            lt = divp.tile([128, 512], F32, tag="lt")
            rec = divp.tile([128, 512], F32, tag="rec")
            ctx_t = divp.tile([128, 512], BF, tag="ctx_t")
            nc.vector.tensor_copy(lt[0:64, :], pv0[64:128, :])
            nc.vector.tensor_copy(lt[64:128, :], pv1[0:64, :])
            nc.vector.reciprocal_approx_fast(rec[:, :], lt[:, :])
            nc.vector.tensor_mul(ctx_t[0:64, :], pv0[0:64, :], rec[0:64, :])
            nc.vector.tensor_mul(ctx_t[64:128, :], pv1[64:128, :], rec[64:128, :])
            pending_outproj.append((ctx_t, b * S + qb * 512))
            for pw in prev_outproj:
                emit_window_outproj(*pw)

        def emit_window_outproj(ctx_t, win):
            for ob in range(8):
                po = ps_proj.tile([128, 512], F32, tag="proj", name=f"po_{win}_{ob}")
                nc.tensor.matmul(
                    po[:],
                    wo_sb[:, ob * 128 : (ob + 1) * 128],
                    ctx_t[:],
                    start=True,
                    stop=True,
                )
                ot = outp.tile([128, 512], BF, tag="ot")
                nc.vector.tensor_copy(ot[:], po[:])
                nc.sync.dma_start(
                    out_ext[ob * 128 : (ob + 1) * 128, win : win + 512], ot[:]
                )

        def flush_outproj():
            while pending_outproj:
                emit_window_outproj(*pending_outproj.pop(0))

        # software pipeline: QKV(0) upfront, then QKV(b+1) interleaved with
        # attention(b) at qb granularity; each projection's stage DMAs are
        # issued one window before its matmuls so the PE never starves.
        # The out-projection of window w is deferred to the end of window
        # w+1 so the PE never waits on the DVE divide chain.
        for which in range(3):
            st = emit_projection_dmas(0, which, nchunks=4 if which == 0 else 2)
            emit_projection_mms(0, which, st)
        next_st = {}
        for b in range(B):
            for qb in range(S // QW):
                if b + 1 < B:
                    if qb == 0:
                        next_st[0] = emit_projection_dmas(b + 1, 0)
                        next_st[1] = emit_projection_dmas(b + 1, 1)
                    elif qb == 1:
                        next_st[2] = emit_projection_dmas(b + 1, 2)
                prev = list(pending_outproj)
                pending_outproj.clear()
                emit_attention_qb(b, qb, prev)
                if b + 1 < B and qb < 3:
                    emit_projection_mms(b + 1, qb, next_st.pop(qb))
        flush_outproj()

    return nc


def _prep_in_maps(inputs):
    q = np.ascontiguousarray(inputs["query"], dtype=np.float32).reshape(R, D)
    k = np.ascontiguousarray(inputs["key"], dtype=np.float32).reshape(R, D)
    v = np.ascontiguousarray(inputs["value"], dtype=np.float32).reshape(R, D)
    Wq = np.asarray(inputs["Wq"], np.float32)
    Wk = np.asarray(inputs["Wk"], np.float32)
    Wv = np.asarray(inputs["Wv"], np.float32)
    Wo = np.asarray(inputs["Wo"], np.float32)
    bq = np.asarray(inputs["bq"], np.float32)
    bk = np.asarray(inputs["bk"], np.float32)
    bv = np.asarray(inputs["bv"], np.float32)

    xqT = np.ascontiguousarray(q.T).astype(bf16)
    xkT = np.ascontiguousarray(k.T).astype(bf16)
    xvT = np.ascontiguousarray(v.T).astype(bf16)
    WqT = np.ascontiguousarray(Wq.T).astype(bf16)
    WkT = np.ascontiguousarray(Wk.T).astype(bf16)
    WvT = np.ascontiguousarray(Wv.T).astype(bf16)
    WoT = np.ascontiguousarray(Wo.T).astype(bf16)
    tri_m = np.arange(128)[:, None] <= np.arange(128)[None, :]
    tri_h = np.ascontiguousarray(
        np.broadcast_to(tri_m[:, None, :], (128, 2, 128))
    ).astype(bf16)
    id_h = np.eye(128, dtype=np.float32).astype(bf16)

    in_maps = []
    for c in range(NCORES):
        sl = slice(c * 128, (c + 1) * 128)
        in_maps.append(
            {
                "xqT": xqT,
                "xkT": xkT,
                "xvT": xvT,
                "wq": np.ascontiguousarray(
                    WqT[:, sl].reshape(8, 128, 128).transpose(1, 0, 2)
                ),
                "wk": np.ascontiguousarray(
                    WkT[:, sl].reshape(8, 128, 128).transpose(1, 0, 2)
                ),
                "wv": np.ascontiguousarray(
                    WvT[:, sl].reshape(8, 128, 128).transpose(1, 0, 2)
                ),
                "wo": np.ascontiguousarray(WoT[sl, :]),
                "bq": np.ascontiguousarray(bq[sl].reshape(128, 1)),
                "bk": np.ascontiguousarray(bk[sl].reshape(128, 1)),
                "bv": np.ascontiguousarray(bv[sl].reshape(128, 1)),
                "tri": tri_h,
                "ident": id_h,
            }
        )
    return in_maps


def kernel(**inputs) -> np.ndarray:
    nc = _CACHE.get("nc")
    if nc is None:
        nc = _build_program()
        nc.finalize()  # Bacc legalization (register alloc, event-sem splitting)
        _CACHE["nc"] = nc
    in_maps = _prep_in_maps(inputs)
    trace = bool(int(os.environ.get("KERNEL_TRACE", "0")))
    res = run_bass_kernel_spmd(nc, in_maps, list(range(NCORES)), trace=trace)
    _CACHE["last"] = res
    acc = res.results[0]["out"].astype(np.float32)
    for c in range(1, NCORES):
        acc += res.results[c]["out"].astype(np.float32)
    full = acc.T + np.asarray(inputs["bo"], np.float32)[None, :]
    return np.ascontiguousarray(full).reshape(B, S, D)
